# revision 16
# baseline (speedup 1.0000x reference)
"""Gaussian RBF network kernel for 8 Trainium2 NeuronCores.

Computes out[n] = sum_c w[c] * exp(-0.5 * (x_n - c_c)^T P (x_n - c_c)),
P = L @ L.T from packed lower-triangular elements, N=8192, C=512, F=128.

Strategy: data-parallel over N (1024 rows per core).  With G = L.T the
exponent is -0.5*||G x - G c||^2 = Gx.Gc - 0.5||Gx||^2 - 0.5||Gc||^2, so
the host precomputes Gx = G @ X.T and Gc = G @ C.T in fp8e4 (the norms
qx/qc are taken of the *rounded* factors, so the exponent stays an exact
negative quadratic form plus ln|w| and can never overflow).  Centers are
sorted w>0 first; qcw[c] = -0.5*qc[c] + ln|w_c|.

Per 128-row tile t (layout [n_partition, c_free]):
  A[n, c]  = Gx[:, n] . Gc[:, c]             (PE, fp8 in, f32 PSUM, K=128)
  A[n, c] += qx[n] + qcw[c]                  (PE fold, K=128: 64 rows carry
             a greedy fp8 row-decomposition of qx[n] against an all-ones
             rhs block, 64 ones rows against a qcw[c] decomposition;
             lattice error <1e-2 absolute vs an O(2500) underflow margin)
  (pairs of tiles share one 2-bank PSUM tile)
  phi      = exp(A)                          (Scalar, one [128,1024] ACT
             per tile PAIR -- amortizes the ~352-cycle pipe fill)
  acc_p/n[t] = sum_c phi over w>0 / w<=0     (tensor_reduce pairs, even
             tiles on DVE, odd tiles on GpSimd, overlapping the next ACT)
A short PE spam loop on memset data runs while the input DMAs are in
flight so the HAM clock manager ramps the PE clock before the real
matmul stream starts.  The raw [128, 16] acc_p|acc_n tile is DMA'd out;
the host does the subtract and the [p, t] -> n = t*128+p transpose.
"""

import contextlib
import ctypes
import sys
import types

import numpy as np

N, C, F = 8192, 512, 128
NCORES = 8
NC = N // NCORES   # rows per core
NT = NC // 128     # 128-row n-tiles per core
KQ = 64            # fp8 rows carrying each of the qx / qcw decompositions
F8MAX = 224.0      # stay below float8_e4m3's 240 finite max
NSPAM = 4          # HAM-warming matmuls issued while input DMAs fly
PAIR = True        # one ACT per 2-tile PSUM pair (False: per-tile ACT)

_cache = {}


def _install_ntff_hook():
    """bass_utils wants antenv.axon_hooks for trace=True under axon; the
    image lacks it. Provide the same ctypes hook trn_boot would install.
    Degrades silently if anything is off (tracing just gets skipped)."""
    if "antenv.axon_hooks" in sys.modules:
        return
    try:
        import antenv

        so_path = "/opt/axon/libaxon_pjrt.so"
        lib = ctypes.CDLL(so_path)
        if not hasattr(lib, "axon_start_nrt_profile"):
            return
        lib.axon_start_nrt_profile.argtypes = [
            ctypes.POINTER(ctypes.c_int64),
            ctypes.c_size_t,
        ]
        lib.axon_start_nrt_profile.restype = ctypes.c_int64
        lib.axon_stop_nrt_profile.argtypes = [ctypes.c_char_p]
        lib.axon_stop_nrt_profile.restype = ctypes.c_int64

        @contextlib.contextmanager
        def _hook(output_dir, device_ids):
            import jax
            import numpy as _np

            # Profiling start fails (rc=-1) until the axon terminal has
            # dispatched at least one computation; warm it with a tiny op.
            d0 = jax.devices()[0]
            x = jax.device_put(_np.ones((2, 2), _np.float32), d0)
            (x + x).block_until_ready()
            if device_ids:
                ids = (ctypes.c_int64 * len(device_ids))(*device_ids)
                rc = lib.axon_start_nrt_profile(ids, len(device_ids))
            else:
                rc = lib.axon_start_nrt_profile(None, 0)
            try:
                yield
            finally:
                if rc == 0:
                    lib.axon_stop_nrt_profile(str(output_dir).encode())

        mod = types.ModuleType("antenv.axon_hooks")
        mod.get_axon_ntff_profile_hook = lambda: _hook
        mod.set_axon_ntff_profile_hook = lambda h: None
        sys.modules["antenv.axon_hooks"] = mod
        antenv.axon_hooks = mod
    except Exception:
        pass


def _build(npos):
    import concourse.bass as bass
    import concourse.mybir as mybir
    import concourse.tile as tile
    from concourse import bacc

    f32 = mybir.dt.float32
    bf16 = mybir.dt.bfloat16
    f8e4 = mybir.dt.float8e4
    Exp = mybir.ActivationFunctionType.Exp
    Alu = mybir.AluOpType
    X_ax = mybir.AxisListType.X

    nc = bacc.Bacc(
        "TRN2", target_bir_lowering=False, debug=False, num_devices=NCORES
    )
    gc_d = nc.dram_tensor("gc", [F, C], f8e4, kind="ExternalInput")
    gx_d = nc.dram_tensor("gx", [F, NC], f8e4, kind="ExternalInput")
    qxs_d = nc.dram_tensor("qxs", [KQ, NT * 128], f8e4, kind="ExternalInput")
    qcs_d = nc.dram_tensor("qcs", [KQ, C], f8e4, kind="ExternalInput")
    # raw acc_p | acc_n; the subtract + transpose happen on the host
    out_d = nc.dram_tensor("out", [F, 2 * NT], f32, kind="ExternalOutput")

    with tile.TileContext(nc) as tc:
        with (
            tc.tile_pool(name="sb", bufs=1) as sb,
            tc.tile_pool(name="phip", bufs=4) as phip,
            tc.tile_pool(name="mm", bufs=4, space=bass.MemorySpace.PSUM) as mm,
        ):
            # ---- fold operand strip [128, NT*128 + C]: columns t*128..
            # hold tile t's fold lhsT ([qx rows ; ones]), the tail C
            # columns hold the shared rhs ([ones ; qcw rows]).  The spread
            # halves come from DRAM (one per queue), the ones via memset.
            fold_sb = sb.tile([F, NT * 128 + C], f8e4)
            nc.gpsimd.memset(fold_sb[KQ:F, 0 : NT * 128], 1.0)
            nc.gpsimd.memset(fold_sb[0:KQ, NT * 128 :], 1.0)
            # ---- PE spam source, memset early so the HAM warmup can run
            # while the input DMAs are still in flight ----
            spam_sb = sb.tile([F, C], f8e4, tag="spam")
            nc.gpsimd.memset(spam_sb[:], 1.0)

            gc_sb = sb.tile([F, C], f8e4)
            nc.sync.dma_start(gc_sb[:], gc_d[:])
            gx_sb = sb.tile([F, NC], f8e4)
            nc.scalar.dma_start(gx_sb[:], gx_d[:])
            nc.sync.dma_start(fold_sb[0:KQ, 0 : NT * 128], qxs_d[:])
            nc.scalar.dma_start(fold_sb[KQ:F, NT * 128 :], qcs_d[:])
            fold_rhs = fold_sb[:, NT * 128 :]

            acc = sb.tile([F, 2 * NT], f32, tag="acc")
            accp = acc[:, 0:NT]
            accn = acc[:, NT : 2 * NT]

            # ---- HAM clock warmup: spam the PE on memset data while the
            # DMAs land, then one real-shape warmup on gc ----
            ps = [
                mm.tile([F, 2 * C], f32, tag="mm", name=f"ps{i}")
                for i in range(NT // 2)
            ]
            for i in range(NSPAM):
                nc.tensor.matmul(
                    ps[0][:, 0:C], spam_sb[:, 0:128], spam_sb[:, 0:C],
                    start=True, stop=True,
                )
            nc.tensor.matmul(
                ps[0][:, 0:C], gc_sb[:, 0:128], gc_sb[:, 0:C],
                start=True, stop=True,
            )

            for p in range(NT // 2):
                a_ps = ps[p]
                acts = [a_ps[:]] if PAIR else [a_ps[:, 0:C], a_ps[:, C : 2 * C]]
                for i in range(2):
                    t = 2 * p + i
                    half = a_ps[:, i * C : (i + 1) * C]
                    nc.tensor.matmul(
                        half, gx_sb[:, t * 128 : (t + 1) * 128], gc_sb[:],
                        start=True, stop=False,
                    )
                    nc.tensor.matmul(
                        half,
                        fold_sb[:, t * 128 : (t + 1) * 128],
                        fold_rhs,
                        start=False, stop=True,
                    )
                phi = phip.tile([F, 2 * C], bf16, tag="phi")
                if PAIR:
                    nc.scalar.activation(phi[:], a_ps[:], Exp)
                else:
                    nc.scalar.activation(phi[:, 0:C], a_ps[:, 0:C], Exp)
                    nc.scalar.activation(phi[:, C : 2 * C], a_ps[:, C : 2 * C], Exp)
                # both tiles of the pair reduce in one 3D-strided op each
                # for the pos and neg column groups: [128, 2, cols] -> [128, 2]
                phi3 = phi[:].rearrange("p (t c) -> p t c", t=2)
                t0 = 2 * p
                if npos > 0:
                    nc.vector.tensor_reduce(
                        accp[:, t0 : t0 + 2], phi3[:, :, 0:npos],
                        axis=X_ax, op=Alu.add,
                    )
                else:
                    nc.vector.memset(accp[:, t0 : t0 + 2], 0.0)
                if npos < C:
                    nc.vector.tensor_reduce(
                        accn[:, t0 : t0 + 2], phi3[:, :, npos:C],
                        axis=X_ax, op=Alu.add,
                    )
                else:
                    nc.vector.memset(accn[:, t0 : t0 + 2], 0.0)

            nc.sync.dma_start(out_d[:], acc[:])

    nc.compile()
    return nc


def _fp8_spread(vals, k, f8):
    """Decompose each value into k fp8 numbers summing to ~it."""
    rem = vals.astype(np.float64).copy()
    rows = np.empty((k,) + vals.shape, f8)
    for i in range(k):
        # even split over the remaining bulk rows, then a geometric
        # mop-up over the last 4 rows (each cuts the residual ~16x)
        div = max(k - 4 - i, 1)
        r = np.clip(rem / div, -F8MAX, F8MAX).astype(np.float32).astype(f8)
        rows[i] = r
        rem -= r.astype(np.float64)
    return rows, rem  # rem = residual error


def _prep_inputs(X, precision_elements, centers, weights):
    import ml_dtypes

    f8e4 = ml_dtypes.float8_e4m3

    ti, tj = np.tril_indices(F)
    L = np.zeros((F, F), np.float32)
    L[ti, tj] = precision_elements
    G = L.T  # exponent = -0.5 ||G x - G c||^2

    Gx8 = (G @ X.astype(np.float32).T).astype(f8e4)  # [F, N]
    Gxr = Gx8.astype(np.float32)
    qx = -0.5 * (Gxr * Gxr).sum(0)  # [N], of the *rounded* factors

    pos = weights > 0
    npos = int(pos.sum())
    perm = np.concatenate([np.nonzero(pos)[0], np.nonzero(~pos)[0]])
    Gc8 = np.ascontiguousarray(
        (G @ centers.astype(np.float32).T)[:, perm]
    ).astype(f8e4)
    Gcr = Gc8.astype(np.float32)
    qc = (Gcr * Gcr).sum(0)  # [C]
    with np.errstate(divide="ignore"):
        lnw = np.log(np.abs(weights[perm].astype(np.float64))).astype(np.float32)
    lnw = np.maximum(lnw, -300.0)
    qcw = -0.5 * qc + lnw

    qcw_rows, qcw_res = _fp8_spread(qcw, KQ, f8e4)
    assert np.abs(qcw_res).max() < 1.0, np.abs(qcw_res).max()
    qcs = np.ascontiguousarray(qcw_rows)

    in_maps = []
    for s in range(NCORES):
        qx_rows, qx_res = _fp8_spread(qx[s * NC : (s + 1) * NC], KQ, f8e4)
        assert np.abs(qx_res).max() < 1.0, np.abs(qx_res).max()
        in_maps.append(
            {
                "gc": Gc8,
                "gx": np.ascontiguousarray(Gx8[:, s * NC : (s + 1) * NC]),
                "qxs": np.ascontiguousarray(qx_rows),
                "qcs": qcs,
            }
        )
    return in_maps, npos


def kernel(X, precision_elements, centers, weights):
    _install_ntff_hook()
    from concourse.bass_utils import run_bass_kernel_spmd

    in_maps, npos = _prep_inputs(X, precision_elements, centers, weights)
    key = ("nc", npos)
    if key not in _cache:
        _cache[key] = _build(npos)
    nc = _cache[key]

    res = run_bass_kernel_spmd(nc, in_maps, core_ids=list(range(NCORES)))
    _cache["last_results"] = res
    outs = []
    for r in res.results:
        acc = np.asarray(r["out"], np.float32)  # [128, 2*NT]: acc_p | acc_n
        outs.append((acc[:, 0:NT] - acc[:, NT:]).T.reshape(NC))
    return np.concatenate(outs).astype(np.float32)


# revision 18
# speedup vs baseline: 1.0386x; 1.0386x over previous
"""Gaussian RBF network kernel for 8 Trainium2 NeuronCores.

Computes out[n] = sum_c w[c] * exp(-0.5 * (x_n - c_c)^T P (x_n - c_c)),
P = L @ L.T from packed lower-triangular elements, N=8192, C=512, F=128.

Strategy: data-parallel over N (1024 rows per core).  With G = L.T the
exponent is -0.5*||G x - G c||^2 = Gx.Gc - 0.5||Gx||^2 - 0.5||Gc||^2, so
the host precomputes Gx = G @ X.T and Gc = G @ C.T in fp8e4 (the norms
qx/qc are taken of the *rounded* factors, so the exponent stays an exact
negative quadratic form plus ln|w| and can never overflow).  Centers are
sorted w>0 first; qcw[c] = -0.5*qc[c] + ln|w_c|.

Per 128-row tile t (layout [n_partition, c_free]):
  A[n, c]  = Gx[:, n] . Gc[:, c]             (PE, fp8 in, f32 PSUM, K=128)
  A[n, c] += qx[n] + qcw[c]                  (PE fold, K=128: 64 rows carry
             a greedy fp8 row-decomposition of qx[n] against an all-ones
             rhs block, 64 ones rows against a qcw[c] decomposition;
             lattice error <1e-2 absolute vs an O(2500) underflow margin)
  (pairs of tiles share one 2-bank PSUM tile)
  phi      = exp(A)                          (Scalar, one [128,1024] ACT
             per tile PAIR -- amortizes the ~352-cycle pipe fill)
  acc_p/n[t] = sum_c phi over w>0 / w<=0     (tensor_reduce pairs, even
             tiles on DVE, odd tiles on GpSimd, overlapping the next ACT)
A short PE spam loop on memset data runs while the input DMAs are in
flight so the HAM clock manager ramps the PE clock before the real
matmul stream starts.  The raw [128, 16] acc_p|acc_n tile is DMA'd out;
the host does the subtract and the [p, t] -> n = t*128+p transpose.
"""

import contextlib
import ctypes
import sys
import types

import numpy as np

N, C, F = 8192, 512, 128
NCORES = 8
NC = N // NCORES   # rows per core
NT = NC // 128     # 128-row n-tiles per core
KQ = 64            # fp8 rows carrying each of the qx / qcw decompositions
F8MAX = 224.0      # stay below float8_e4m3's 240 finite max
NSPAM = 3          # HAM-warming matmuls issued while input DMAs fly
PAIR = True        # one ACT per 2-tile PSUM pair (False: per-tile ACT)

_cache = {}


def _install_ntff_hook():
    """bass_utils wants antenv.axon_hooks for trace=True under axon; the
    image lacks it. Provide the same ctypes hook trn_boot would install.
    Degrades silently if anything is off (tracing just gets skipped)."""
    if "antenv.axon_hooks" in sys.modules:
        return
    try:
        import antenv

        so_path = "/opt/axon/libaxon_pjrt.so"
        lib = ctypes.CDLL(so_path)
        if not hasattr(lib, "axon_start_nrt_profile"):
            return
        lib.axon_start_nrt_profile.argtypes = [
            ctypes.POINTER(ctypes.c_int64),
            ctypes.c_size_t,
        ]
        lib.axon_start_nrt_profile.restype = ctypes.c_int64
        lib.axon_stop_nrt_profile.argtypes = [ctypes.c_char_p]
        lib.axon_stop_nrt_profile.restype = ctypes.c_int64

        @contextlib.contextmanager
        def _hook(output_dir, device_ids):
            import jax
            import numpy as _np

            # Profiling start fails (rc=-1) until the axon terminal has
            # dispatched at least one computation; warm it with a tiny op.
            d0 = jax.devices()[0]
            x = jax.device_put(_np.ones((2, 2), _np.float32), d0)
            (x + x).block_until_ready()
            if device_ids:
                ids = (ctypes.c_int64 * len(device_ids))(*device_ids)
                rc = lib.axon_start_nrt_profile(ids, len(device_ids))
            else:
                rc = lib.axon_start_nrt_profile(None, 0)
            try:
                yield
            finally:
                if rc == 0:
                    lib.axon_stop_nrt_profile(str(output_dir).encode())

        mod = types.ModuleType("antenv.axon_hooks")
        mod.get_axon_ntff_profile_hook = lambda: _hook
        mod.set_axon_ntff_profile_hook = lambda h: None
        sys.modules["antenv.axon_hooks"] = mod
        antenv.axon_hooks = mod
    except Exception:
        pass


def _build(npos):
    import concourse.bass as bass
    import concourse.mybir as mybir
    import concourse.tile as tile
    from concourse import bacc

    f32 = mybir.dt.float32
    bf16 = mybir.dt.bfloat16
    f8e4 = mybir.dt.float8e4
    Exp = mybir.ActivationFunctionType.Exp
    Alu = mybir.AluOpType
    X_ax = mybir.AxisListType.X

    nc = bacc.Bacc(
        "TRN2", target_bir_lowering=False, debug=False, num_devices=NCORES
    )
    gc_d = nc.dram_tensor("gc", [F, C], f8e4, kind="ExternalInput")
    gx_d = nc.dram_tensor("gx", [F, NC], f8e4, kind="ExternalInput")
    qxs_d = nc.dram_tensor("qxs", [KQ, NT * 128], f8e4, kind="ExternalInput")
    qcs_d = nc.dram_tensor("qcs", [KQ, C], f8e4, kind="ExternalInput")
    # raw acc_p | acc_n; the subtract + transpose happen on the host
    out_d = nc.dram_tensor("out", [F, 2 * NT], f32, kind="ExternalOutput")

    with tile.TileContext(nc) as tc:
        with (
            tc.tile_pool(name="sb", bufs=1) as sb,
            tc.tile_pool(name="phip", bufs=4) as phip,
            tc.tile_pool(name="mm", bufs=4, space=bass.MemorySpace.PSUM) as mm,
        ):
            # ---- PE spam source, memset early (DVE dispatches fast) so
            # the HAM clock warmup runs while the input DMAs are in
            # flight; the FD=512 spams also warm the real matmul config ----
            spam_sb = sb.tile([F, C], f8e4, tag="spam")
            nc.vector.memset(spam_sb[:], 1.0)

            # ---- fold operand strip [128, NT*128 + C]: columns t*128..
            # hold tile t's fold lhsT ([qx rows ; ones]), the tail C
            # columns hold the shared rhs ([ones ; qcw rows]).  The spread
            # halves come from DRAM (one per queue), the ones via memset.
            fold_sb = sb.tile([F, NT * 128 + C], f8e4)
            nc.gpsimd.memset(fold_sb[KQ:F, 0 : NT * 128], 1.0)
            nc.gpsimd.memset(fold_sb[0:KQ, NT * 128 :], 1.0)

            # need-ordered loads: the fold spreads gate the start=True fold
            # matmuls, so they go first on their queues; gx tiles 0-3 land
            # before the gc + gx tail.
            gx_sb = sb.tile([F, NC], f8e4)
            gc_sb = sb.tile([F, C], f8e4)
            nc.sync.dma_start(fold_sb[KQ:F, NT * 128 :], qcs_d[:])
            nc.scalar.dma_start(gx_sb[:, 0 : NC // 2], gx_d[:, 0 : NC // 2])
            nc.sync.dma_start(fold_sb[0:KQ, 0 : NT * 128], qxs_d[:])
            nc.scalar.dma_start(gc_sb[:], gc_d[:])
            nc.scalar.dma_start(gx_sb[:, NC // 2 :], gx_d[:, NC // 2 :])
            fold_rhs = fold_sb[:, NT * 128 :]

            acc = sb.tile([F, 2 * NT], f32, tag="acc")
            accp = acc[:, 0:NT]
            accn = acc[:, NT : 2 * NT]

            ps = [
                mm.tile([F, 2 * C], f32, tag="mm", name=f"ps{i}")
                for i in range(NT // 2)
            ]
            # HAM clock warmup on the memset data while the DMAs land
            for i in range(NSPAM):
                nc.tensor.matmul(
                    ps[0][:, 0:C], spam_sb[:, 0:128], spam_sb[:, 0:C],
                    start=True, stop=True,
                )

            for p in range(NT // 2):
                a_ps = ps[p]
                # fold first (start=True): it only needs the small spread
                # DMAs, so it runs before gc/gx land
                for i in range(2):
                    t = 2 * p + i
                    half = a_ps[:, i * C : (i + 1) * C]
                    nc.tensor.matmul(
                        half,
                        fold_sb[:, t * 128 : (t + 1) * 128],
                        fold_rhs,
                        start=True, stop=False,
                    )
                for i in range(2):
                    t = 2 * p + i
                    half = a_ps[:, i * C : (i + 1) * C]
                    nc.tensor.matmul(
                        half, gx_sb[:, t * 128 : (t + 1) * 128], gc_sb[:],
                        start=False, stop=True,
                    )
                phi = phip.tile([F, 2 * C], bf16, tag="phi")
                if PAIR:
                    nc.scalar.activation(phi[:], a_ps[:], Exp)
                else:
                    nc.scalar.activation(phi[:, 0:C], a_ps[:, 0:C], Exp)
                    nc.scalar.activation(phi[:, C : 2 * C], a_ps[:, C : 2 * C], Exp)
                # both tiles of the pair reduce in one 3D-strided op each
                # for the pos and neg column groups: [128, 2, cols] -> [128, 2]
                phi3 = phi[:].rearrange("p (t c) -> p t c", t=2)
                t0 = 2 * p
                if npos > 0:
                    nc.vector.tensor_reduce(
                        accp[:, t0 : t0 + 2], phi3[:, :, 0:npos],
                        axis=X_ax, op=Alu.add,
                    )
                else:
                    nc.vector.memset(accp[:, t0 : t0 + 2], 0.0)
                if npos < C:
                    nc.vector.tensor_reduce(
                        accn[:, t0 : t0 + 2], phi3[:, :, npos:C],
                        axis=X_ax, op=Alu.add,
                    )
                else:
                    nc.vector.memset(accn[:, t0 : t0 + 2], 0.0)

            nc.sync.dma_start(out_d[:], acc[:])

    nc.compile()
    return nc


def _fp8_spread(vals, k, f8):
    """Decompose each value into k fp8 numbers summing to ~it."""
    rem = vals.astype(np.float64).copy()
    rows = np.empty((k,) + vals.shape, f8)
    for i in range(k):
        # even split over the remaining bulk rows, then a geometric
        # mop-up over the last 4 rows (each cuts the residual ~16x)
        div = max(k - 4 - i, 1)
        r = np.clip(rem / div, -F8MAX, F8MAX).astype(np.float32).astype(f8)
        rows[i] = r
        rem -= r.astype(np.float64)
    return rows, rem  # rem = residual error


def _prep_inputs(X, precision_elements, centers, weights):
    import ml_dtypes

    f8e4 = ml_dtypes.float8_e4m3

    ti, tj = np.tril_indices(F)
    L = np.zeros((F, F), np.float32)
    L[ti, tj] = precision_elements
    G = L.T  # exponent = -0.5 ||G x - G c||^2

    Gx8 = (G @ X.astype(np.float32).T).astype(f8e4)  # [F, N]
    Gxr = Gx8.astype(np.float32)
    qx = -0.5 * (Gxr * Gxr).sum(0)  # [N], of the *rounded* factors

    pos = weights > 0
    npos = int(pos.sum())
    perm = np.concatenate([np.nonzero(pos)[0], np.nonzero(~pos)[0]])
    Gc8 = np.ascontiguousarray(
        (G @ centers.astype(np.float32).T)[:, perm]
    ).astype(f8e4)
    Gcr = Gc8.astype(np.float32)
    qc = (Gcr * Gcr).sum(0)  # [C]
    with np.errstate(divide="ignore"):
        lnw = np.log(np.abs(weights[perm].astype(np.float64))).astype(np.float32)
    lnw = np.maximum(lnw, -300.0)
    qcw = -0.5 * qc + lnw

    qcw_rows, qcw_res = _fp8_spread(qcw, KQ, f8e4)
    assert np.abs(qcw_res).max() < 1.0, np.abs(qcw_res).max()
    qcs = np.ascontiguousarray(qcw_rows)

    in_maps = []
    for s in range(NCORES):
        qx_rows, qx_res = _fp8_spread(qx[s * NC : (s + 1) * NC], KQ, f8e4)
        assert np.abs(qx_res).max() < 1.0, np.abs(qx_res).max()
        in_maps.append(
            {
                "gc": Gc8,
                "gx": np.ascontiguousarray(Gx8[:, s * NC : (s + 1) * NC]),
                "qxs": np.ascontiguousarray(qx_rows),
                "qcs": qcs,
            }
        )
    return in_maps, npos


def kernel(X, precision_elements, centers, weights):
    _install_ntff_hook()
    from concourse.bass_utils import run_bass_kernel_spmd

    in_maps, npos = _prep_inputs(X, precision_elements, centers, weights)
    key = ("nc", npos)
    if key not in _cache:
        _cache[key] = _build(npos)
    nc = _cache[key]

    res = run_bass_kernel_spmd(nc, in_maps, core_ids=list(range(NCORES)))
    _cache["last_results"] = res
    outs = []
    for r in res.results:
        acc = np.asarray(r["out"], np.float32)  # [128, 2*NT]: acc_p | acc_n
        outs.append((acc[:, 0:NT] - acc[:, NT:]).T.reshape(NC))
    return np.concatenate(outs).astype(np.float32)


# revision 23
# speedup vs baseline: 1.0403x; 1.0017x over previous
"""Gaussian RBF network kernel for 8 Trainium2 NeuronCores.

Computes out[n] = sum_c w[c] * exp(-0.5 * (x_n - c_c)^T P (x_n - c_c)),
P = L @ L.T from packed lower-triangular elements, N=8192, C=512, F=128.

Strategy: data-parallel over N (1024 rows per core).  With G = L.T the
exponent is -0.5*||G x - G c||^2 = Gx.Gc - 0.5||Gx||^2 - 0.5||Gc||^2, so
the host precomputes Gx = G @ X.T and Gc = G @ C.T in fp8e4 (the norms
qx/qc are taken of the *rounded* factors, so the exponent stays an exact
negative quadratic form plus ln|w| and can never overflow).  Centers are
sorted w>0 first; qcw[c] = -0.5*qc[c] + ln|w_c|.

Per 128-row tile t (layout [n_partition, c_free]):
  A[n, c]  = Gx[:, n] . Gc[:, c]             (PE, fp8 in, f32 PSUM, K=128)
  A[n, c] += qx[n] + qcw[c]                  (PE fold, K=128: 64 rows carry
             a greedy fp8 row-decomposition of qx[n] against an all-ones
             rhs block, 64 ones rows against a qcw[c] decomposition;
             lattice error <1e-2 absolute vs an O(2500) underflow margin)
  (pairs of tiles share one 2-bank PSUM tile)
  phi      = exp(A)                          (Scalar, one [128,1024] ACT
             per tile PAIR -- amortizes the ~352-cycle pipe fill)
  acc_p/n[t] = sum_c phi over w>0 / w<=0     (tensor_reduce pairs, even
             tiles on DVE, odd tiles on GpSimd, overlapping the next ACT)
A short PE spam loop on memset data runs while the input DMAs are in
flight so the HAM clock manager ramps the PE clock before the real
matmul stream starts.  The raw [128, 16] acc_p|acc_n tile is DMA'd out;
the host does the subtract and the [p, t] -> n = t*128+p transpose.
"""

import contextlib
import ctypes
import sys
import types

import numpy as np

N, C, F = 8192, 512, 128
NCORES = 8
NC = N // NCORES   # rows per core
NT = NC // 128     # 128-row n-tiles per core
KQ = 64            # fp8 rows carrying each of the qx / qcw decompositions
F8MAX = 224.0      # stay below float8_e4m3's 240 finite max
NSPAM = 3          # HAM-warming matmuls issued while input DMAs fly
PAIR = True        # one ACT per 2-tile PSUM pair (False: per-tile ACT)

_cache = {}


def _install_ntff_hook():
    """bass_utils wants antenv.axon_hooks for trace=True under axon; the
    image lacks it. Provide the same ctypes hook trn_boot would install.
    Degrades silently if anything is off (tracing just gets skipped)."""
    if "antenv.axon_hooks" in sys.modules:
        return
    try:
        import antenv

        so_path = "/opt/axon/libaxon_pjrt.so"
        lib = ctypes.CDLL(so_path)
        if not hasattr(lib, "axon_start_nrt_profile"):
            return
        lib.axon_start_nrt_profile.argtypes = [
            ctypes.POINTER(ctypes.c_int64),
            ctypes.c_size_t,
        ]
        lib.axon_start_nrt_profile.restype = ctypes.c_int64
        lib.axon_stop_nrt_profile.argtypes = [ctypes.c_char_p]
        lib.axon_stop_nrt_profile.restype = ctypes.c_int64

        @contextlib.contextmanager
        def _hook(output_dir, device_ids):
            import jax
            import numpy as _np

            # Profiling start fails (rc=-1) until the axon terminal has
            # dispatched at least one computation; warm it with a tiny op.
            d0 = jax.devices()[0]
            x = jax.device_put(_np.ones((2, 2), _np.float32), d0)
            (x + x).block_until_ready()
            if device_ids:
                ids = (ctypes.c_int64 * len(device_ids))(*device_ids)
                rc = lib.axon_start_nrt_profile(ids, len(device_ids))
            else:
                rc = lib.axon_start_nrt_profile(None, 0)
            try:
                yield
            finally:
                if rc == 0:
                    lib.axon_stop_nrt_profile(str(output_dir).encode())

        mod = types.ModuleType("antenv.axon_hooks")
        mod.get_axon_ntff_profile_hook = lambda: _hook
        mod.set_axon_ntff_profile_hook = lambda h: None
        sys.modules["antenv.axon_hooks"] = mod
        antenv.axon_hooks = mod
    except Exception:
        pass


def _build(npos):
    import concourse.bass as bass
    import concourse.mybir as mybir
    import concourse.tile as tile
    from concourse import bacc

    f32 = mybir.dt.float32
    bf16 = mybir.dt.bfloat16
    f8e4 = mybir.dt.float8e4
    Exp = mybir.ActivationFunctionType.Exp
    Alu = mybir.AluOpType
    X_ax = mybir.AxisListType.X

    nc = bacc.Bacc(
        "TRN2", target_bir_lowering=False, debug=False, num_devices=NCORES
    )
    gc_d = nc.dram_tensor("gc", [F, C], f8e4, kind="ExternalInput")
    gx_d = nc.dram_tensor("gx", [F, NC], f8e4, kind="ExternalInput")
    # qx spreads (cols 0:NT*128) | qcw spreads (cols NT*128:) in one tensor
    qs_d = nc.dram_tensor("qs", [KQ, NT * 128 + C], f8e4, kind="ExternalInput")
    # raw acc_p | acc_n; the subtract + transpose happen on the host
    out_d = nc.dram_tensor("out", [F, 2 * NT], f32, kind="ExternalOutput")

    with tile.TileContext(nc) as tc:
        with (
            tc.tile_pool(name="sb", bufs=1) as sb,
            tc.tile_pool(name="phip", bufs=4) as phip,
            tc.tile_pool(name="mm", bufs=4, space=bass.MemorySpace.PSUM) as mm,
        ):
            # ---- PE spam source, memset early (DVE dispatches fast) so
            # the HAM clock warmup runs while the input DMAs are in
            # flight; the FD=512 spams also warm the real matmul config ----
            spam_sb = sb.tile([F, C], f8e4, tag="spam")
            nc.vector.memset(spam_sb[:], 1.0)

            # ---- fold operand strip [128, NT*128 + C]: columns t*128..
            # hold tile t's fold lhsT ([qx rows ; ones]), the tail C
            # columns hold the shared rhs ([ones ; qcw rows]).  The spread
            # halves come from DRAM (one per queue), the ones via memset.
            fold_sb = sb.tile([F, NT * 128 + C], f8e4)
            nc.gpsimd.memset(fold_sb[KQ:F, 0 : NT * 128], 1.0)
            nc.gpsimd.memset(fold_sb[0:KQ, NT * 128 :], 1.0)

            # need-ordered loads: the fold spreads gate the start=True fold
            # matmuls, so they go first on their queue; gx tiles 0-3 land
            # before the gc + gx tail.  The qx spread lands in SBUF rows
            # 0:KQ, the qcw spread in rows KQ:2KQ -- one DMA, two regions.
            gx_sb = sb.tile([F, NC], f8e4)
            gc_sb = sb.tile([F, C], f8e4)
            nc.sync.dma_start(fold_sb[0:KQ, 0 : NT * 128], qs_d[:, 0 : NT * 128])
            nc.scalar.dma_start(gx_sb[:, 0 : NC // 2], gx_d[:, 0 : NC // 2])
            nc.sync.dma_start(fold_sb[KQ:F, NT * 128 :], qs_d[:, NT * 128 :])
            nc.scalar.dma_start(gc_sb[:], gc_d[:])
            nc.scalar.dma_start(gx_sb[:, NC // 2 :], gx_d[:, NC // 2 :])
            fold_rhs = fold_sb[:, NT * 128 :]

            acc = sb.tile([F, 2 * NT], f32, tag="acc")
            accp = acc[:, 0:NT]
            accn = acc[:, NT : 2 * NT]

            ps = [
                mm.tile([F, 2 * C], f32, tag="mm", name=f"ps{i}")
                for i in range(NT // 2)
            ]
            # HAM clock warmup on the memset data while the DMAs land;
            # the trailing half-width spam fills the gap until the fold
            # spreads arrive without delaying the first real matmul
            for i in range(NSPAM):
                nc.tensor.matmul(
                    ps[0][:, 0:C], spam_sb[:, 0:128], spam_sb[:, 0:C],
                    start=True, stop=True,
                )
            nc.tensor.matmul(
                ps[0][:, 0 : C // 2], spam_sb[:, 0:128], spam_sb[:, 0 : C // 2],
                start=True, stop=True,
            )

            for p in range(NT // 2):
                a_ps = ps[p]
                # fold first (start=True): it only needs the small spread
                # DMAs, so it runs before gc/gx land
                for i in range(2):
                    t = 2 * p + i
                    half = a_ps[:, i * C : (i + 1) * C]
                    nc.tensor.matmul(
                        half,
                        fold_sb[:, t * 128 : (t + 1) * 128],
                        fold_rhs,
                        start=True, stop=False,
                    )
                for i in range(2):
                    t = 2 * p + i
                    half = a_ps[:, i * C : (i + 1) * C]
                    nc.tensor.matmul(
                        half, gx_sb[:, t * 128 : (t + 1) * 128], gc_sb[:],
                        start=False, stop=True,
                    )
                phi = phip.tile([F, 2 * C], bf16, tag="phi")
                last = p == NT // 2 - 1
                if PAIR and not last:
                    # one 2-bank ACT per pair; both tiles then reduce in one
                    # 3D-strided op each for the pos/neg column groups:
                    # [128, 2, cols] -> [128, 2]
                    nc.scalar.activation(phi[:], a_ps[:], Exp)
                    phi3 = phi[:].rearrange("p (t c) -> p t c", t=2)
                    t0 = 2 * p
                    if npos > 0:
                        nc.vector.tensor_reduce(
                            accp[:, t0 : t0 + 2], phi3[:, :, 0:npos],
                            axis=X_ax, op=Alu.add,
                        )
                    else:
                        nc.vector.memset(accp[:, t0 : t0 + 2], 0.0)
                    if npos < C:
                        nc.vector.tensor_reduce(
                            accn[:, t0 : t0 + 2], phi3[:, :, npos:C],
                            axis=X_ax, op=Alu.add,
                        )
                    else:
                        nc.vector.memset(accn[:, t0 : t0 + 2], 0.0)
                else:
                    # last pair runs as two singles so tile 6's reduces
                    # overlap tile 7's ACT and only tile 7's small 2D
                    # reduces trail the final exp
                    for i in range(2):
                        t = 2 * p + i
                        lo = i * C
                        nc.scalar.activation(
                            phi[:, lo : lo + C], a_ps[:, lo : lo + C], Exp
                        )
                        if npos > 0:
                            nc.vector.tensor_reduce(
                                accp[:, t : t + 1], phi[:, lo : lo + npos],
                                axis=X_ax, op=Alu.add,
                            )
                        else:
                            nc.vector.memset(accp[:, t : t + 1], 0.0)
                        if npos < C:
                            nc.vector.tensor_reduce(
                                accn[:, t : t + 1], phi[:, lo + npos : lo + C],
                                axis=X_ax, op=Alu.add,
                            )
                        else:
                            nc.vector.memset(accn[:, t : t + 1], 0.0)

            nc.sync.dma_start(out_d[:], acc[:])

    nc.compile()
    return nc


def _fp8_spread(vals, k, f8):
    """Decompose each value into k fp8 numbers summing to ~it."""
    rem = vals.astype(np.float64).copy()
    rows = np.empty((k,) + vals.shape, f8)
    for i in range(k):
        # even split over the remaining bulk rows, then a geometric
        # mop-up over the last 4 rows (each cuts the residual ~16x)
        div = max(k - 4 - i, 1)
        r = np.clip(rem / div, -F8MAX, F8MAX).astype(np.float32).astype(f8)
        rows[i] = r
        rem -= r.astype(np.float64)
    return rows, rem  # rem = residual error


def _prep_inputs(X, precision_elements, centers, weights):
    import ml_dtypes

    f8e4 = ml_dtypes.float8_e4m3

    ti, tj = np.tril_indices(F)
    L = np.zeros((F, F), np.float32)
    L[ti, tj] = precision_elements
    G = L.T  # exponent = -0.5 ||G x - G c||^2

    Gx8 = (G @ X.astype(np.float32).T).astype(f8e4)  # [F, N]
    Gxr = Gx8.astype(np.float32)
    qx = -0.5 * (Gxr * Gxr).sum(0)  # [N], of the *rounded* factors

    pos = weights > 0
    npos = int(pos.sum())
    perm = np.concatenate([np.nonzero(pos)[0], np.nonzero(~pos)[0]])
    Gc8 = np.ascontiguousarray(
        (G @ centers.astype(np.float32).T)[:, perm]
    ).astype(f8e4)
    Gcr = Gc8.astype(np.float32)
    qc = (Gcr * Gcr).sum(0)  # [C]
    with np.errstate(divide="ignore"):
        lnw = np.log(np.abs(weights[perm].astype(np.float64))).astype(np.float32)
    lnw = np.maximum(lnw, -300.0)
    qcw = -0.5 * qc + lnw

    qcw_rows, qcw_res = _fp8_spread(qcw, KQ, f8e4)
    assert np.abs(qcw_res).max() < 1.0, np.abs(qcw_res).max()

    in_maps = []
    for s in range(NCORES):
        qx_rows, qx_res = _fp8_spread(qx[s * NC : (s + 1) * NC], KQ, f8e4)
        assert np.abs(qx_res).max() < 1.0, np.abs(qx_res).max()
        qs = np.empty((KQ, NC + C), f8e4)
        qs[:, 0:NC] = qx_rows
        qs[:, NC:] = qcw_rows
        in_maps.append(
            {
                "gc": Gc8,
                "gx": np.ascontiguousarray(Gx8[:, s * NC : (s + 1) * NC]),
                "qs": qs,
            }
        )
    return in_maps, npos


def kernel(X, precision_elements, centers, weights):
    _install_ntff_hook()
    from concourse.bass_utils import run_bass_kernel_spmd

    in_maps, npos = _prep_inputs(X, precision_elements, centers, weights)
    key = ("nc", npos)
    if key not in _cache:
        _cache[key] = _build(npos)
    nc = _cache[key]

    res = run_bass_kernel_spmd(nc, in_maps, core_ids=list(range(NCORES)))
    _cache["last_results"] = res
    outs = []
    for r in res.results:
        acc = np.asarray(r["out"], np.float32)  # [128, 2*NT]: acc_p | acc_n
        outs.append((acc[:, 0:NT] - acc[:, NT:]).T.reshape(NC))
    return np.concatenate(outs).astype(np.float32)
